# revision 22
# baseline (speedup 1.0000x reference)
"""Causal attention (B=8, S=2048, D=768, single head) on 8 trn2 NeuronCores.

Sharding: data-parallel over batch — core b computes batch element b.

v2: all matmul operands in bf16 (FWL fast weight loads, 2x DVE, half
SBUF traffic); wq/wk/wv loaded once at start (f32 staged on two DMA
queues, cast to persistent bf16 tiles) instead of re-streamed per
chunk; per-chunk phase order T->Q->K->S->V->AV so the exp of the last
diagonal score block hides behind the V-projection phase.

Per-core pipeline (fused over 512-wide s-chunks):
  1. x chunk DMA (f32) -> cast bf16 (gpsimd) -> PE transpose -> xT
  2. QT/KT [e_part, s] = wq/wk col-block.T @ xT; V [s_part, e] = xT.T @ wv
  3. scores^T tile [k_part, q] = KT_blk.T @ QT_chunk (contract e)
  4. exp (scale fused) on ACT -> PT bf16; triangular mask on corner block
  5. out[q, e] (+denominator via ones cols of V) = PT_blk.T @ V_blk
  6. normalize via reciprocal (DVE) + scaled copy (ACT), DMA out
"""

import numpy as np

import concourse.bass as bass
import concourse.mybir as mybir
from concourse import bacc
from concourse.tile import TileContext
from concourse.bass_utils import run_bass_kernel_spmd
from concourse.masks import make_identity

B, S, D = 8, 2048, 768
P = 128
ND = D // P            # 6 feature blocks
NB = S // P            # 16 seq blocks
CH = 512               # s-chunk width
NCH = S // CH          # 4 chunks
QPC = CH // P          # 4 q/s-blocks per chunk
SCALE = 1.0 / float(np.sqrt(D))
F32 = mybir.dt.float32
F32R = mybir.dt.float32r
BF16 = mybir.dt.bfloat16
EXP = mybir.ActivationFunctionType.Exp


def _build_nc():
    nc = bacc.Bacc(None, target_bir_lowering=False)
    xb = nc.dram_tensor("xb", [S, D], F32, kind="ExternalInput")
    wq_d = nc.dram_tensor("wq", [D, D], F32, kind="ExternalInput")
    wk_d = nc.dram_tensor("wk", [D, D], F32, kind="ExternalInput")
    wv_d = nc.dram_tensor("wv", [D, D], F32, kind="ExternalInput")
    out_d = nc.dram_tensor("out", [S, D], F32, kind="ExternalOutput")

    # [d, e] weight views as [d_in(128), d_block(6), e(768)]
    wq_r = wq_d[:, :].rearrange("(o p) e -> p o e", p=P)
    wk_r = wk_d[:, :].rearrange("(o p) e -> p o e", p=P)
    wv_r = wv_d[:, :].rearrange("(o p) e -> p o e", p=P)

    with TileContext(nc) as tc:
        with (
            tc.tile_pool(name="const", bufs=1) as constp,
            tc.tile_pool(name="persist", bufs=1) as persist,
            tc.tile_pool(name="wstage", bufs=6) as wstage,
            tc.tile_pool(name="xload", bufs=4) as xload,
            tc.tile_pool(name="xt", bufs=2) as xtp,
            tc.tile_pool(name="qt", bufs=2) as qtp,
            tc.tile_pool(name="outp", bufs=2) as outp,
            tc.tile_pool(name="rc", bufs=4) as rcp,
            tc.tile_pool(name="psW", bufs=4, space="PSUM") as psW,
            tc.tile_pool(name="psO", bufs=4, space="PSUM") as psO,
        ):
            ident = constp.tile([P, P], F32)
            make_identity(nc, ident)
            ident_r = constp.tile([P, P], F32R)
            nc.vector.tensor_copy(ident_r, ident)
            # tri[p, j] = 1.0 if p <= j else 0.0 (keep k <= q on the
            # diagonal 128x128 corner of each score block)
            tri = constp.tile([P, P], BF16)
            nc.gpsimd.memset(tri, 1.0)
            nc.gpsimd.affine_select(
                out=tri,
                in_=tri,
                compare_op=mybir.AluOpType.is_ge,
                fill=0.0,
                base=0,
                pattern=[[1, P]],
                channel_multiplier=-1,
            )

            WQ = persist.tile([P, ND, D], BF16)
            WK = persist.tile([P, ND, D], BF16)
            WV = persist.tile([P, ND, D], BF16)
            KT = persist.tile([P, ND, S], BF16)      # K^T: [e_in, eo, s]
            V = persist.tile([P, NB, D + 2], BF16)   # [s_in, sb, e]; cols D..D+1 = 1.0
            PT = persist.tile([P, NB, CH], BF16)     # exp(scores^T) blocks of chunk
            ones_col = constp.tile([P, NB, 2], BF16)
            nc.vector.memset(ones_col, 1.0)
            nc.vector.tensor_copy(V[:, :, D : D + 2], ones_col)

            # ---- startup. DMA rings are strict FIFO per engine, so order
            # by need-time: x chunk0 split across sync+scalar, then wq in
            # 6 o-pieces alternating rings, then 2 wk pieces; the other 4
            # wk pieces + wv ride the gpsimd software-DGE queue (3rd
            # channel). Casts go on DVE (wq, after chunk-0 xT copies) and
            # ACT (wk, wv — first exp isn't needed until ~21us).
            H = ND // 2
            xf_tiles = []
            for sb in range(QPC):
                xf = xload.tile([P, D], F32R, tag="xf")
                eng = nc.sync if sb < 2 else nc.scalar
                eng.dma_start(xf, xb[sb * P : (sb + 1) * P, :].bitcast(F32R))
                xf_tiles.append(xf)
            sv0 = wstage.tile([P, H, D], F32, tag="wv")
            nc.gpsimd.dma_start(sv0, wv_r[:, 0:H, :])
            sv1 = wstage.tile([P, H, D], F32, tag="wv")
            nc.gpsimd.dma_start(sv1, wv_r[:, H:ND, :])
            wq_stage = []
            for o in range(ND):
                s = wstage.tile([P, 1, D], F32, tag="ws")
                eng = nc.scalar if o % 2 == 0 else nc.sync
                eng.dma_start(s, wq_r[:, o : o + 1, :])
                wq_stage.append(s)
            # wk tail pieces ride the gpsimd DGE queue behind wv
            wk_stage = []
            for o in range(ND):
                s = wstage.tile([P, 1, D], F32, tag="ws")
                eng = [nc.scalar, nc.sync, nc.scalar, nc.sync, nc.gpsimd, nc.gpsimd][o]
                eng.dma_start(s, wk_r[:, o : o + 1, :])
                wk_stage.append(s)

            for o in range(ND):
                nc.scalar.copy(WK[:, o : o + 1, :], wk_stage[o])
            nc.scalar.copy(WV[:, 0:H, :], sv0)
            nc.scalar.copy(WV[:, H:ND, :], sv1)

            def emit_transpose(c, xT):
                # f32r transpose straight from the staged f32 x tiles; the
                # PSUM->SBUF copy casts to bf16.
                for sb in range(QPC):
                    xr = xf_tiles[sb]
                    for h in range(2):
                        ps_t = psW.tile([P, 3 * P], F32R, tag="w")
                        for dh in range(3):
                            do = h * 3 + dh
                            nc.tensor.transpose(
                                ps_t[:, dh * P : (dh + 1) * P],
                                xr[:, do * P : (do + 1) * P],
                                ident_r,
                            )
                        nc.vector.tensor_copy(
                            xT[:, h * 3 : h * 3 + 3, sb * P : (sb + 1) * P],
                            ps_t.bitcast(F32).rearrange("p (o s) -> p o s", o=3),
                        )

            xT = xtp.tile([P, ND, CH], BF16, tag="xT")
            emit_transpose(0, xT)

            for c in range(NCH):
                if c == 0:
                    # wq casts on DVE after chunk-0's xT copies
                    for o in range(ND):
                        nc.vector.tensor_copy(WQ[:, o : o + 1, :], wq_stage[o])

                # ---- Q^T then K^T projections for this chunk
                QT = qtp.tile([P, ND, CH], BF16, tag="qt")
                for eb in range(ND):
                    pq = psW.tile([P, CH], F32, tag="w")
                    for do in range(ND):
                        nc.tensor.matmul(
                            pq,
                            WQ[:, do, eb * P : (eb + 1) * P],
                            xT[:, do, :],
                            start=(do == 0),
                            stop=(do == ND - 1),
                        )
                    nc.vector.tensor_copy(QT[:, eb, :], pq)
                    # prefetch next chunk's x during the Q phase
                    if c + 1 < NCH and eb < QPC:
                        sb = eb
                        xf = xload.tile([P, D], F32R, tag="xf")
                        s0 = ((c + 1) * QPC + sb) * P
                        nc.sync.dma_start(xf, xb[s0 : s0 + P, :].bitcast(F32R))
                        xf_tiles[sb] = xf
                for eb in range(ND):
                    pk = psW.tile([P, CH], F32, tag="w")
                    for do in range(ND):
                        nc.tensor.matmul(
                            pk,
                            WK[:, do, eb * P : (eb + 1) * P],
                            xT[:, do, :],
                            start=(do == 0),
                            stop=(do == ND - 1),
                        )
                    nc.vector.tensor_copy(KT[:, eb, c * CH : (c + 1) * CH], pk)

                # ---- scores^T + exp; triangular mask on the diagonal corner
                for kb in range(QPC * (c + 1)):
                    i = kb - QPC * c
                    q0 = max(i, 0) * P
                    W = CH - q0
                    ps_s = psW.tile([P, CH], F32, tag="w")
                    for eo in range(ND):
                        nc.tensor.matmul(
                            ps_s[:, 0:W],
                            KT[:, eo, kb * P : (kb + 1) * P],
                            QT[:, eo, q0:CH],
                            start=(eo == 0),
                            stop=(eo == ND - 1),
                        )
                    nc.scalar.activation(PT[:, kb, q0:CH], ps_s[:, 0:W], EXP, scale=SCALE)
                    if i >= 0:
                        nc.vector.tensor_mul(
                            PT[:, kb, q0 : q0 + P], PT[:, kb, q0 : q0 + P], tri
                        )

                # ---- V projection for this chunk (after S: exp hides here)
                for sb in range(QPC):
                    xt_blk = xT[:, :, sb * P : (sb + 1) * P]
                    pv0 = psW.tile([P, CH], F32, tag="w")
                    for do in range(ND):
                        nc.tensor.matmul(
                            pv0,
                            xt_blk[:, do, :],
                            WV[:, do, 0:CH],
                            start=(do == 0),
                            stop=(do == ND - 1),
                        )
                    nc.scalar.copy(V[:, c * QPC + sb, 0:CH], pv0)
                    pv1 = psW.tile([P, CH], F32, tag="w")
                    for do in range(ND):
                        nc.tensor.matmul(
                            pv1[:, 0 : D - CH],
                            xt_blk[:, do, :],
                            WV[:, do, CH:D],
                            start=(do == 0),
                            stop=(do == ND - 1),
                        )
                    nc.scalar.copy(V[:, c * QPC + sb, CH:D], pv1[:, 0 : D - CH])
                    # next chunk's transposes mid-V-phase, so xT(c+1) and
                    # its DVE copies finish well before AV(c) ends
                    if sb == 2 and c + 1 < NCH:
                        xT_next = xtp.tile([P, ND, CH], BF16, tag="xT")
                        emit_transpose(c + 1, xT_next)
                if c + 1 == NCH:
                    xT_next = None

                # ---- attn @ [V | 1], normalize, store
                for qs in range(QPC):
                    qb = c * QPC + qs
                    po0 = psO.tile([P, CH], F32, tag="o")
                    po1 = psO.tile([P, D + 2 - CH], F32, tag="o")
                    for kb in range(qb + 1):
                        lhs = PT[:, kb, qs * P : (qs + 1) * P]
                        nc.tensor.matmul(
                            po0,
                            lhs,
                            V[:, kb, 0:CH],
                            start=(kb == 0),
                            stop=(kb == qb),
                        )
                        nc.tensor.matmul(
                            po1,
                            lhs,
                            V[:, kb, CH : D + 2],
                            start=(kb == 0),
                            stop=(kb == qb),
                        )
                    recip = rcp.tile([P, 1], F32, tag="rc")
                    nc.vector.reciprocal(recip, po1[:, D - CH : D - CH + 1])
                    o_sb = outp.tile([P, D], F32, tag="o")
                    nc.scalar.mul(o_sb[:, 0:CH], po0, recip)
                    nc.sync.dma_start(out_d[qb * P : (qb + 1) * P, 0:CH], o_sb[:, 0:CH])
                    nc.scalar.mul(o_sb[:, CH:D], po1[:, 0 : D - CH], recip)
                    nc.sync.dma_start(out_d[qb * P : (qb + 1) * P, CH:D], o_sb[:, CH:D])

                xT = xT_next

    nc.finalize()
    return nc


_NC_CACHE = None


def _get_nc():
    global _NC_CACHE
    if _NC_CACHE is None:
        _NC_CACHE = _build_nc()
    return _NC_CACHE


def run(inputs, trace=False):
    x = np.asarray(inputs["x"], dtype=np.float32)
    wq = np.asarray(inputs["wq"], dtype=np.float32)
    wk = np.asarray(inputs["wk"], dtype=np.float32)
    wv = np.asarray(inputs["wv"], dtype=np.float32)
    nc = _get_nc()
    in_maps = [
        {"xb": np.ascontiguousarray(x[b]), "wq": wq, "wk": wk, "wv": wv}
        for b in range(B)
    ]
    res = run_bass_kernel_spmd(nc, in_maps, core_ids=list(range(B)), trace=trace)
    out = np.stack([r["out"] for r in res.results]).astype(np.float32)
    return out, res


def kernel(x, wq, wk, wv):
    out, _ = run({"x": x, "wq": wq, "wk": wk, "wv": wv}, trace=False)
    return out


# revision 23
# speedup vs baseline: 1.0031x; 1.0031x over previous
"""Causal attention (B=8, S=2048, D=768, single head) on 8 trn2 NeuronCores.

Sharding: data-parallel over batch — core b computes batch element b.

v2: all matmul operands in bf16 (FWL fast weight loads, 2x DVE, half
SBUF traffic); wq/wk/wv loaded once at start (f32 staged on two DMA
queues, cast to persistent bf16 tiles) instead of re-streamed per
chunk; per-chunk phase order T->Q->K->S->V->AV so the exp of the last
diagonal score block hides behind the V-projection phase.

Per-core pipeline (fused over 512-wide s-chunks):
  1. x chunk DMA (f32) -> cast bf16 (gpsimd) -> PE transpose -> xT
  2. QT/KT [e_part, s] = wq/wk col-block.T @ xT; V [s_part, e] = xT.T @ wv
  3. scores^T tile [k_part, q] = KT_blk.T @ QT_chunk (contract e)
  4. exp (scale fused) on ACT -> PT bf16; triangular mask on corner block
  5. out[q, e] (+denominator via ones cols of V) = PT_blk.T @ V_blk
  6. normalize via reciprocal (DVE) + scaled copy (ACT), DMA out
"""

import numpy as np

import concourse.bass as bass
import concourse.mybir as mybir
from concourse import bacc
from concourse.tile import TileContext
from concourse.bass_utils import run_bass_kernel_spmd
from concourse.masks import make_identity

B, S, D = 8, 2048, 768
P = 128
ND = D // P            # 6 feature blocks
NB = S // P            # 16 seq blocks
CH = 512               # s-chunk width
NCH = S // CH          # 4 chunks
QPC = CH // P          # 4 q/s-blocks per chunk
SCALE = 1.0 / float(np.sqrt(D))
F32 = mybir.dt.float32
F32R = mybir.dt.float32r
BF16 = mybir.dt.bfloat16
EXP = mybir.ActivationFunctionType.Exp


def _build_nc():
    nc = bacc.Bacc(None, target_bir_lowering=False)
    xb = nc.dram_tensor("xb", [S, D], F32, kind="ExternalInput")
    wq_d = nc.dram_tensor("wq", [D, D], F32, kind="ExternalInput")
    wk_d = nc.dram_tensor("wk", [D, D], F32, kind="ExternalInput")
    wv_d = nc.dram_tensor("wv", [D, D], F32, kind="ExternalInput")
    out_d = nc.dram_tensor("out", [S, D], F32, kind="ExternalOutput")

    # [d, e] weight views as [d_in(128), d_block(6), e(768)]
    wq_r = wq_d[:, :].rearrange("(o p) e -> p o e", p=P)
    wk_r = wk_d[:, :].rearrange("(o p) e -> p o e", p=P)
    wv_r = wv_d[:, :].rearrange("(o p) e -> p o e", p=P)

    with TileContext(nc) as tc:
        with (
            tc.tile_pool(name="const", bufs=1) as constp,
            tc.tile_pool(name="persist", bufs=1) as persist,
            tc.tile_pool(name="wstage", bufs=6) as wstage,
            tc.tile_pool(name="xload", bufs=4) as xload,
            tc.tile_pool(name="xt", bufs=2) as xtp,
            tc.tile_pool(name="qt", bufs=2) as qtp,
            tc.tile_pool(name="outp", bufs=2) as outp,
            tc.tile_pool(name="rc", bufs=4) as rcp,
            tc.tile_pool(name="psW", bufs=4, space="PSUM") as psW,
            tc.tile_pool(name="psO", bufs=4, space="PSUM") as psO,
        ):
            ident = constp.tile([P, P], F32)
            make_identity(nc, ident)
            ident_r = constp.tile([P, P], F32R)
            nc.vector.tensor_copy(ident_r, ident)
            # tri[p, j] = 1.0 if p <= j else 0.0 (keep k <= q on the
            # diagonal 128x128 corner of each score block)
            tri = constp.tile([P, P], BF16)
            nc.gpsimd.memset(tri, 1.0)
            nc.gpsimd.affine_select(
                out=tri,
                in_=tri,
                compare_op=mybir.AluOpType.is_ge,
                fill=0.0,
                base=0,
                pattern=[[1, P]],
                channel_multiplier=-1,
            )

            WQ = persist.tile([P, ND, D], BF16)
            WK = persist.tile([P, ND, D], BF16)
            WV = persist.tile([P, ND, D], BF16)
            KT = persist.tile([P, ND, S], BF16)      # K^T: [e_in, eo, s]
            V = persist.tile([P, NB, D + 2], BF16)   # [s_in, sb, e]; cols D..D+1 = 1.0
            PT = persist.tile([P, NB, CH], BF16)     # exp(scores^T) blocks of chunk
            ones_col = constp.tile([P, NB, 2], BF16)
            nc.vector.memset(ones_col, 1.0)
            nc.vector.tensor_copy(V[:, :, D : D + 2], ones_col)

            # ---- startup. DMA rings are strict FIFO per engine, so order
            # by need-time: x chunk0 split across sync+scalar, then wq in
            # 6 o-pieces alternating rings, then 2 wk pieces; the other 4
            # wk pieces + wv ride the gpsimd software-DGE queue (3rd
            # channel). Casts go on DVE (wq, after chunk-0 xT copies) and
            # ACT (wk, wv — first exp isn't needed until ~21us).
            H = ND // 2
            xf_tiles = []
            for sb in range(QPC):
                xf = xload.tile([P, D], F32R, tag="xf")
                eng = nc.sync if sb < 2 else nc.scalar
                eng.dma_start(xf, xb[sb * P : (sb + 1) * P, :].bitcast(F32R))
                xf_tiles.append(xf)
            sv0 = wstage.tile([P, H, D], F32, tag="wv")
            nc.gpsimd.dma_start(sv0, wv_r[:, 0:H, :])
            sv1 = wstage.tile([P, H, D], F32, tag="wv")
            nc.gpsimd.dma_start(sv1, wv_r[:, H:ND, :])
            wq_stage = []
            for o in range(ND):
                s = wstage.tile([P, 1, D], F32, tag="ws")
                eng = nc.scalar if o % 2 == 0 else nc.sync
                eng.dma_start(s, wq_r[:, o : o + 1, :])
                wq_stage.append(s)
            wk_stage = []
            for o in range(ND):
                s = wstage.tile([P, 1, D], F32, tag="ws")
                eng = nc.scalar if o % 2 == 0 else nc.sync
                eng.dma_start(s, wk_r[:, o : o + 1, :])
                wk_stage.append(s)

            for o in range(ND):
                nc.scalar.copy(WK[:, o : o + 1, :], wk_stage[o])
            nc.scalar.copy(WV[:, 0:H, :], sv0)
            nc.scalar.copy(WV[:, H:ND, :], sv1)

            def emit_transpose(c, xT):
                # f32r transpose straight from the staged f32 x tiles; the
                # PSUM->SBUF copy casts to bf16.
                for sb in range(QPC):
                    xr = xf_tiles[sb]
                    for h in range(2):
                        ps_t = psW.tile([P, 3 * P], F32R, tag="w")
                        for dh in range(3):
                            do = h * 3 + dh
                            nc.tensor.transpose(
                                ps_t[:, dh * P : (dh + 1) * P],
                                xr[:, do * P : (do + 1) * P],
                                ident_r,
                            )
                        nc.vector.tensor_copy(
                            xT[:, h * 3 : h * 3 + 3, sb * P : (sb + 1) * P],
                            ps_t.bitcast(F32).rearrange("p (o s) -> p o s", o=3),
                        )

            xT = xtp.tile([P, ND, CH], BF16, tag="xT")
            emit_transpose(0, xT)

            for c in range(NCH):
                if c == 0:
                    # wq casts on DVE after chunk-0's xT copies
                    for o in range(ND):
                        nc.vector.tensor_copy(WQ[:, o : o + 1, :], wq_stage[o])

                # ---- Q^T then K^T projections for this chunk
                QT = qtp.tile([P, ND, CH], BF16, tag="qt")
                for eb in range(ND):
                    pq = psW.tile([P, CH], F32, tag="w")
                    for do in range(ND):
                        nc.tensor.matmul(
                            pq,
                            WQ[:, do, eb * P : (eb + 1) * P],
                            xT[:, do, :],
                            start=(do == 0),
                            stop=(do == ND - 1),
                        )
                    nc.vector.tensor_copy(QT[:, eb, :], pq)
                    # prefetch next chunk's x during the Q phase
                    if c + 1 < NCH and eb < QPC:
                        sb = eb
                        xf = xload.tile([P, D], F32R, tag="xf")
                        s0 = ((c + 1) * QPC + sb) * P
                        nc.sync.dma_start(xf, xb[s0 : s0 + P, :].bitcast(F32R))
                        xf_tiles[sb] = xf
                for eb in range(ND):
                    pk = psW.tile([P, CH], F32, tag="w")
                    for do in range(ND):
                        nc.tensor.matmul(
                            pk,
                            WK[:, do, eb * P : (eb + 1) * P],
                            xT[:, do, :],
                            start=(do == 0),
                            stop=(do == ND - 1),
                        )
                    nc.vector.tensor_copy(KT[:, eb, c * CH : (c + 1) * CH], pk)

                # ---- scores^T + exp; triangular mask on the diagonal corner
                for kb in range(QPC * (c + 1)):
                    i = kb - QPC * c
                    q0 = max(i, 0) * P
                    W = CH - q0
                    ps_s = psW.tile([P, CH], F32, tag="w")
                    for eo in range(ND):
                        nc.tensor.matmul(
                            ps_s[:, 0:W],
                            KT[:, eo, kb * P : (kb + 1) * P],
                            QT[:, eo, q0:CH],
                            start=(eo == 0),
                            stop=(eo == ND - 1),
                        )
                    nc.scalar.activation(PT[:, kb, q0:CH], ps_s[:, 0:W], EXP, scale=SCALE)
                    if i >= 0:
                        nc.vector.tensor_mul(
                            PT[:, kb, q0 : q0 + P], PT[:, kb, q0 : q0 + P], tri
                        )

                # ---- V projection for this chunk (after S: exp hides here)
                for sb in range(QPC):
                    xt_blk = xT[:, :, sb * P : (sb + 1) * P]
                    pv0 = psW.tile([P, CH], F32, tag="w")
                    for do in range(ND):
                        nc.tensor.matmul(
                            pv0,
                            xt_blk[:, do, :],
                            WV[:, do, 0:CH],
                            start=(do == 0),
                            stop=(do == ND - 1),
                        )
                    nc.scalar.copy(V[:, c * QPC + sb, 0:CH], pv0)
                    pv1 = psW.tile([P, CH], F32, tag="w")
                    for do in range(ND):
                        nc.tensor.matmul(
                            pv1[:, 0 : D - CH],
                            xt_blk[:, do, :],
                            WV[:, do, CH:D],
                            start=(do == 0),
                            stop=(do == ND - 1),
                        )
                    nc.scalar.copy(V[:, c * QPC + sb, CH:D], pv1[:, 0 : D - CH])
                    # next chunk's transposes mid-V-phase, so xT(c+1) and
                    # its DVE copies finish well before AV(c) ends
                    if sb == 2 and c + 1 < NCH:
                        xT_next = xtp.tile([P, ND, CH], BF16, tag="xT")
                        emit_transpose(c + 1, xT_next)
                if c + 1 == NCH:
                    xT_next = None

                # ---- attn @ [V | 1], normalize, store
                for qs in range(QPC):
                    qb = c * QPC + qs
                    po0 = psO.tile([P, CH], F32, tag="o")
                    po1 = psO.tile([P, D + 2 - CH], F32, tag="o")
                    for kb in range(qb + 1):
                        lhs = PT[:, kb, qs * P : (qs + 1) * P]
                        nc.tensor.matmul(
                            po0,
                            lhs,
                            V[:, kb, 0:CH],
                            start=(kb == 0),
                            stop=(kb == qb),
                        )
                        nc.tensor.matmul(
                            po1,
                            lhs,
                            V[:, kb, CH : D + 2],
                            start=(kb == 0),
                            stop=(kb == qb),
                        )
                    recip = rcp.tile([P, 1], F32, tag="rc")
                    nc.vector.reciprocal(recip, po1[:, D - CH : D - CH + 1])
                    o_sb = outp.tile([P, D], F32, tag="o")
                    nc.scalar.mul(o_sb[:, 0:CH], po0, recip)
                    nc.sync.dma_start(out_d[qb * P : (qb + 1) * P, 0:CH], o_sb[:, 0:CH])
                    nc.scalar.mul(o_sb[:, CH:D], po1[:, 0 : D - CH], recip)
                    nc.sync.dma_start(out_d[qb * P : (qb + 1) * P, CH:D], o_sb[:, CH:D])

                xT = xT_next

    nc.finalize()
    return nc


_NC_CACHE = None


def _get_nc():
    global _NC_CACHE
    if _NC_CACHE is None:
        _NC_CACHE = _build_nc()
    return _NC_CACHE


def run(inputs, trace=False):
    x = np.asarray(inputs["x"], dtype=np.float32)
    wq = np.asarray(inputs["wq"], dtype=np.float32)
    wk = np.asarray(inputs["wk"], dtype=np.float32)
    wv = np.asarray(inputs["wv"], dtype=np.float32)
    nc = _get_nc()
    in_maps = [
        {"xb": np.ascontiguousarray(x[b]), "wq": wq, "wk": wk, "wv": wv}
        for b in range(B)
    ]
    res = run_bass_kernel_spmd(nc, in_maps, core_ids=list(range(B)), trace=trace)
    out = np.stack([r["out"] for r in res.results]).astype(np.float32)
    return out, res


def kernel(x, wq, wk, wv):
    out, _ = run({"x": x, "wq": wq, "wk": wk, "wv": wv}, trace=False)
    return out


# revision 26
# speedup vs baseline: 1.0300x; 1.0267x over previous
"""Causal attention (B=8, S=2048, D=768, single head) on 8 trn2 NeuronCores.

Sharding: data-parallel over batch — core b computes batch element b.

v2: all matmul operands in bf16 (FWL fast weight loads, 2x DVE, half
SBUF traffic); wq/wk/wv loaded once at start (f32 staged on two DMA
queues, cast to persistent bf16 tiles) instead of re-streamed per
chunk; per-chunk phase order T->Q->K->S->V->AV so the exp of the last
diagonal score block hides behind the V-projection phase.

Per-core pipeline (fused over 512-wide s-chunks):
  1. x chunk DMA (f32) -> cast bf16 (gpsimd) -> PE transpose -> xT
  2. QT/KT [e_part, s] = wq/wk col-block.T @ xT; V [s_part, e] = xT.T @ wv
  3. scores^T tile [k_part, q] = KT_blk.T @ QT_chunk (contract e)
  4. exp (scale fused) on ACT -> PT bf16; triangular mask on corner block
  5. out[q, e] (+denominator via ones cols of V) = PT_blk.T @ V_blk
  6. normalize via reciprocal (DVE) + scaled copy (ACT), DMA out
"""

import numpy as np

import concourse.bass as bass
import concourse.mybir as mybir
from concourse import bacc
from concourse.tile import TileContext
from concourse.bass_utils import run_bass_kernel_spmd
from concourse.masks import make_identity

B, S, D = 8, 2048, 768
P = 128
ND = D // P            # 6 feature blocks
NB = S // P            # 16 seq blocks
CH = 512               # s-chunk width
NCH = S // CH          # 4 chunks
QPC = CH // P          # 4 q/s-blocks per chunk
SCALE = 1.0 / float(np.sqrt(D))
F32 = mybir.dt.float32
F32R = mybir.dt.float32r
BF16 = mybir.dt.bfloat16
EXP = mybir.ActivationFunctionType.Exp


def _build_nc():
    nc = bacc.Bacc(None, target_bir_lowering=False)
    xb = nc.dram_tensor("xb", [S, D], F32, kind="ExternalInput")
    wq_d = nc.dram_tensor("wq", [D, D], F32, kind="ExternalInput")
    wk_d = nc.dram_tensor("wk", [D, D], F32, kind="ExternalInput")
    wv_d = nc.dram_tensor("wv", [D, D], F32, kind="ExternalInput")
    out_d = nc.dram_tensor("out", [S, D], F32, kind="ExternalOutput")

    # [d, e] weight views as [d_in(128), d_block(6), e(768)]
    wq_r = wq_d[:, :].rearrange("(o p) e -> p o e", p=P)
    wk_r = wk_d[:, :].rearrange("(o p) e -> p o e", p=P)
    wv_r = wv_d[:, :].rearrange("(o p) e -> p o e", p=P)

    with TileContext(nc) as tc:
        with (
            tc.tile_pool(name="const", bufs=1) as constp,
            tc.tile_pool(name="persist", bufs=1) as persist,
            tc.tile_pool(name="wstage", bufs=4) as wstage,
            tc.tile_pool(name="xload", bufs=4) as xload,
            tc.tile_pool(name="qt", bufs=2) as qtp,
            tc.tile_pool(name="outp", bufs=2) as outp,
            tc.tile_pool(name="rc", bufs=4) as rcp,
            tc.tile_pool(name="psW", bufs=4, space="PSUM") as psW,
            tc.tile_pool(name="psO", bufs=4, space="PSUM") as psO,
        ):
            ident = constp.tile([P, P], F32)
            make_identity(nc, ident)
            ident_r = constp.tile([P, P], F32R)
            nc.vector.tensor_copy(ident_r, ident)
            ident_b = constp.tile([P, P], BF16)
            nc.vector.tensor_copy(ident_b, ident)
            # tri[p, j] = 1.0 if p <= j else 0.0 (keep k <= q on the
            # diagonal 128x128 corner of each score block)
            tri = constp.tile([P, P], BF16)
            nc.gpsimd.memset(tri, 1.0)
            nc.gpsimd.affine_select(
                out=tri,
                in_=tri,
                compare_op=mybir.AluOpType.is_ge,
                fill=0.0,
                base=0,
                pattern=[[1, P]],
                channel_multiplier=-1,
            )

            WQ = persist.tile([P, ND, D], BF16)
            WK = persist.tile([P, ND, D], BF16)
            WV = persist.tile([P, ND, D], BF16)
            # M = Wq @ Wk^T lets chunks 1-3 skip the K projection:
            # scores^T = xT.T @ B with B = (x M)^T, so only B (one
            # projection) is needed per chunk instead of Q^T and K^T.
            WqT = persist.tile([P, ND, D], BF16)     # Wq^T: [e_in, eo, d]
            WkT = persist.tile([P, ND, D], BF16)     # Wk^T: [e_in, eo, d]
            Mt = persist.tile([P, ND, D], BF16)      # M:    [d'_in, o', d]
            KT = persist.tile([P, ND, CH], BF16)     # chunk-0 K^T only
            XT = persist.tile([P, ND, S], BF16)      # x^T, all chunks
            V = persist.tile([P, NB, D + 2], BF16)   # [s_in, sb, e]; cols D..D+1 = 1.0
            PT = persist.tile([P, NB, CH], BF16)     # exp(scores^T) blocks of chunk
            ones_col = constp.tile([P, NB, 2], BF16)
            nc.vector.memset(ones_col, 1.0)
            nc.vector.tensor_copy(V[:, :, D : D + 2], ones_col)

            # ---- startup. DMA rings are strict FIFO per engine, so order
            # by need-time: x chunk0 split across sync+scalar, then wq in
            # 6 o-pieces alternating rings, then 2 wk pieces; the other 4
            # wk pieces + wv ride the gpsimd software-DGE queue (3rd
            # channel). Casts go on DVE (wq, after chunk-0 xT copies) and
            # ACT (wk, wv — first exp isn't needed until ~21us).
            H = ND // 2
            xf_tiles = []
            for sb in range(QPC):
                xf = xload.tile([P, D], F32R, tag="xf")
                eng = nc.sync if sb < 2 else nc.scalar
                eng.dma_start(xf, xb[sb * P : (sb + 1) * P, :].bitcast(F32R))
                xf_tiles.append(xf)
            sv0 = wstage.tile([P, H, D], F32, tag="wv")
            nc.gpsimd.dma_start(sv0, wv_r[:, 0:H, :])
            sv1 = wstage.tile([P, H, D], F32, tag="wv")
            nc.gpsimd.dma_start(sv1, wv_r[:, H:ND, :])
            wq_stage = []
            for o in range(ND):
                s = wstage.tile([P, 1, D], F32, tag="ws")
                eng = nc.scalar if o % 2 == 0 else nc.sync
                eng.dma_start(s, wq_r[:, o : o + 1, :])
                wq_stage.append(s)
            wk_stage = []
            for o in range(ND):
                s = wstage.tile([P, 1, D], F32, tag="ws")
                eng = nc.scalar if o % 2 == 0 else nc.sync
                eng.dma_start(s, wk_r[:, o : o + 1, :])
                wk_stage.append(s)

            for o in range(ND):
                nc.scalar.copy(WK[:, o : o + 1, :], wk_stage[o])
            nc.scalar.copy(WV[:, 0:H, :], sv0)
            nc.scalar.copy(WV[:, H:ND, :], sv1)

            def emit_transpose(c):
                # f32r transpose straight from the staged f32 x tiles; the
                # PSUM->SBUF copy casts to bf16 into XT's chunk-c slice.
                for sb in range(QPC):
                    xr = xf_tiles[sb]
                    s0 = c * CH + sb * P
                    for h in range(2):
                        ps_t = psW.tile([P, 3 * P], F32R, tag="w")
                        for dh in range(3):
                            do = h * 3 + dh
                            nc.tensor.transpose(
                                ps_t[:, dh * P : (dh + 1) * P],
                                xr[:, do * P : (do + 1) * P],
                                ident_r,
                            )
                        nc.vector.tensor_copy(
                            XT[:, h * 3 : h * 3 + 3, s0 : s0 + P],
                            ps_t.bitcast(F32).rearrange("p (o s) -> p o s", o=3),
                        )

            def emit_wtrans(W_t, WT_t, o):
                # transpose one staged weight piece [d-block o, e] into
                # WT_t[:, :, o-block] (bf16 PE transpose + one DVE copy)
                ps_w = psW.tile([P, ND * P], BF16, tag="w")
                for eo in range(ND):
                    nc.tensor.transpose(
                        ps_w[:, eo * P : (eo + 1) * P],
                        W_t[:, o, eo * P : (eo + 1) * P],
                        ident_b,
                    )
                nc.vector.tensor_copy(
                    WT_t[:, :, o * P : (o + 1) * P],
                    ps_w.rearrange("p (o s) -> p o s", o=ND),
                )

            emit_transpose(0)

            for c in range(NCH):
                if c == 0:
                    # wq casts on DVE after chunk-0's xT copies
                    for o in range(ND):
                        nc.vector.tensor_copy(WQ[:, o : o + 1, :], wq_stage[o])

                # ---- projection: chunk 0 computes Q^T and K^T the direct
                # way (M isn't ready yet); chunks 1-3 compute only
                # B = (x M)^T, which replaces both.
                QT = qtp.tile([P, ND, CH], BF16, tag="qt")
                lhsW = WQ if c == 0 else Mt
                for eb in range(ND):
                    pq = psW.tile([P, CH], F32, tag="w")
                    for do in range(ND):
                        nc.tensor.matmul(
                            pq,
                            lhsW[:, do, eb * P : (eb + 1) * P],
                            XT[:, do, c * CH : (c + 1) * CH],
                            start=(do == 0),
                            stop=(do == ND - 1),
                        )
                    nc.vector.tensor_copy(QT[:, eb, :], pq)
                    if c == 0:
                        # fill the wq-piece DMA-wait bubbles: transpose the
                        # arrived wq piece for the M computation
                        emit_wtrans(WQ, WqT, eb)
                    # prefetch next chunk's x during this phase
                    if c + 1 < NCH and eb < QPC:
                        sb = eb
                        xf = xload.tile([P, D], F32R, tag="xf")
                        s0 = ((c + 1) * QPC + sb) * P
                        nc.sync.dma_start(xf, xb[s0 : s0 + P, :].bitcast(F32R))
                        xf_tiles[sb] = xf
                if c == 0:
                    for eb in range(ND):
                        pk = psW.tile([P, CH], F32, tag="w")
                        for do in range(ND):
                            nc.tensor.matmul(
                                pk,
                                WK[:, do, eb * P : (eb + 1) * P],
                                XT[:, do, 0:CH],
                                start=(do == 0),
                                stop=(do == ND - 1),
                            )
                        nc.vector.tensor_copy(KT[:, eb, :], pk)
                        emit_wtrans(WK, WkT, eb)
                        # M(a, b-pair) once both wk pieces of the pair are
                        # transposed: M = Wq @ Wk^T, contract e
                        if eb % 2 == 1:
                            b2 = eb // 2
                            for a in range(ND):
                                pm = psW.tile([P, 2 * P], F32, tag="w")
                                for eo in range(ND):
                                    nc.tensor.matmul(
                                        pm,
                                        WqT[:, eo, a * P : (a + 1) * P],
                                        WkT[:, eo, b2 * 2 * P : (b2 + 1) * 2 * P],
                                        start=(eo == 0),
                                        stop=(eo == ND - 1),
                                    )
                                nc.vector.tensor_copy(
                                    Mt[:, a, b2 * 2 * P : (b2 + 1) * 2 * P], pm
                                )

                # ---- scores^T + exp; triangular mask on the diagonal corner
                for kb in range(QPC * (c + 1)):
                    i = kb - QPC * c
                    q0 = max(i, 0) * P
                    W = CH - q0
                    ps_s = psW.tile([P, CH], F32, tag="w")
                    lhsS = KT if c == 0 else XT
                    for eo in range(ND):
                        nc.tensor.matmul(
                            ps_s[:, 0:W],
                            lhsS[:, eo, kb * P : (kb + 1) * P],
                            QT[:, eo, q0:CH],
                            start=(eo == 0),
                            stop=(eo == ND - 1),
                        )
                    nc.scalar.activation(PT[:, kb, q0:CH], ps_s[:, 0:W], EXP, scale=SCALE)
                    if i >= 0:
                        nc.vector.tensor_mul(
                            PT[:, kb, q0 : q0 + P], PT[:, kb, q0 : q0 + P], tri
                        )

                # ---- V projection for this chunk (after S: exp hides here)
                for sb in range(QPC):
                    xt_blk = XT[:, :, (c * QPC + sb) * P : (c * QPC + sb + 1) * P]
                    pv0 = psW.tile([P, CH], F32, tag="w")
                    for do in range(ND):
                        nc.tensor.matmul(
                            pv0,
                            xt_blk[:, do, :],
                            WV[:, do, 0:CH],
                            start=(do == 0),
                            stop=(do == ND - 1),
                        )
                    nc.scalar.copy(V[:, c * QPC + sb, 0:CH], pv0)
                    pv1 = psW.tile([P, CH], F32, tag="w")
                    for do in range(ND):
                        nc.tensor.matmul(
                            pv1[:, 0 : D - CH],
                            xt_blk[:, do, :],
                            WV[:, do, CH:D],
                            start=(do == 0),
                            stop=(do == ND - 1),
                        )
                    nc.scalar.copy(V[:, c * QPC + sb, CH:D], pv1[:, 0 : D - CH])
                    # next chunk's transposes mid-V-phase, so XT(c+1) and
                    # its DVE copies finish well before AV(c) ends
                    if sb == 2 and c + 1 < NCH:
                        emit_transpose(c + 1)

                # ---- attn @ [V | 1], normalize, store
                for qs in range(QPC):
                    qb = c * QPC + qs
                    po0 = psO.tile([P, CH], F32, tag="o")
                    po1 = psO.tile([P, D + 2 - CH], F32, tag="o")
                    for kb in range(qb + 1):
                        lhs = PT[:, kb, qs * P : (qs + 1) * P]
                        nc.tensor.matmul(
                            po0,
                            lhs,
                            V[:, kb, 0:CH],
                            start=(kb == 0),
                            stop=(kb == qb),
                        )
                        nc.tensor.matmul(
                            po1,
                            lhs,
                            V[:, kb, CH : D + 2],
                            start=(kb == 0),
                            stop=(kb == qb),
                        )
                    recip = rcp.tile([P, 1], F32, tag="rc")
                    nc.vector.reciprocal(recip, po1[:, D - CH : D - CH + 1])
                    o_sb = outp.tile([P, D], F32, tag="o")
                    nc.scalar.mul(o_sb[:, 0:CH], po0, recip)
                    nc.sync.dma_start(out_d[qb * P : (qb + 1) * P, 0:CH], o_sb[:, 0:CH])
                    nc.scalar.mul(o_sb[:, CH:D], po1[:, 0 : D - CH], recip)
                    nc.sync.dma_start(out_d[qb * P : (qb + 1) * P, CH:D], o_sb[:, CH:D])


    nc.finalize()
    return nc


_NC_CACHE = None


def _get_nc():
    global _NC_CACHE
    if _NC_CACHE is None:
        _NC_CACHE = _build_nc()
    return _NC_CACHE


def run(inputs, trace=False):
    x = np.asarray(inputs["x"], dtype=np.float32)
    wq = np.asarray(inputs["wq"], dtype=np.float32)
    wk = np.asarray(inputs["wk"], dtype=np.float32)
    wv = np.asarray(inputs["wv"], dtype=np.float32)
    nc = _get_nc()
    in_maps = [
        {"xb": np.ascontiguousarray(x[b]), "wq": wq, "wk": wk, "wv": wv}
        for b in range(B)
    ]
    res = run_bass_kernel_spmd(nc, in_maps, core_ids=list(range(B)), trace=trace)
    out = np.stack([r["out"] for r in res.results]).astype(np.float32)
    return out, res


def kernel(x, wq, wk, wv):
    out, _ = run({"x": x, "wq": wq, "wk": wk, "wv": wv}, trace=False)
    return out
